# revision 4
# baseline (speedup 1.0000x reference)
import sys
import numpy as np

sys.path.insert(0, '/opt/trn_rl_repo')
import ml_dtypes  # noqa: E402

BF16 = ml_dtypes.bfloat16

B, CIN, H, W = 16, 512, 96, 96
HEAD = 4
QK = 512
CV = 256
S = H * W            # 9216
SCALE = (QK // HEAD) ** -0.5
EPS = 1e-5
NCORES = 8
BLOC = B // NCORES   # 2 images per core
ST = 288             # spatial tile = 3 rows of 96
NJ = 3               # attention j's per strip
NST = S // ST        # 32
NEC = 576            # epilogue chunk (phase B)

_COMPILED = None


def _build():
    import concourse.bass as bass
    import concourse.mybir as mybir
    import concourse.tile as tile
    import concourse.tile_utils as tile_utils
    from concourse import bacc
    from concourse.alu_op_type import AluOpType

    tile_utils.max_sbuf_usage = 206 * 1024

    f32 = mybir.dt.float32
    f32r = mybir.dt.float32r
    bf16 = mybir.dt.bfloat16

    nc = bacc.Bacc("TRN2", target_bir_lowering=False, debug=False, enable_asserts=False)

    x_d = nc.dram_tensor("x", [BLOC * CIN, S], f32r, kind="ExternalInput")
    wqt_d = nc.dram_tensor("wqt", [256, 512], f32r, kind="ExternalInput")
    wkt_d = nc.dram_tensor("wkt", [256, 512], f32r, kind="ExternalInput")
    wvt_d = nc.dram_tensor("wvt", [4, 128, 256], f32r, kind="ExternalInput")
    bq_d = nc.dram_tensor("bq", [512], f32, kind="ExternalInput")
    bk_d = nc.dram_tensor("bk", [512], f32, kind="ExternalInput")
    ones_d = nc.dram_tensor("ones64", [64], bf16, kind="ExternalInput")
    out_d = nc.dram_tensor("out", [BLOC * CIN, S], bf16, kind="ExternalOutput")
    zr_d = nc.dram_tensor("zr", [BLOC * 256, S], bf16, kind="ExternalOutput")
    vt_scr = nc.dram_tensor("vt_scr", [S, 256], bf16)

    with tile.TileContext(nc) as tc:
        import contextlib
        with contextlib.ExitStack() as ctx:
            singles = ctx.enter_context(tc.tile_pool(name="singles", bufs=1))
            bigs = ctx.enter_context(tc.tile_pool(name="bigs", bufs=1))
            xs = ctx.enter_context(tc.tile_pool(name="xs", bufs=2))
            strips = ctx.enter_context(tc.tile_pool(name="strips", bufs=2))
            outs = ctx.enter_context(tc.tile_pool(name="outs", bufs=2))
            ps_conv = ctx.enter_context(tc.tile_pool(name="ps_conv", bufs=2, space="PSUM"))
            ps_vt = ctx.enter_context(tc.tile_pool(name="ps_vt", bufs=2, space="PSUM"))
            ps_e = ctx.enter_context(tc.tile_pool(name="ps_e", bufs=2, space="PSUM"))
            ps_av = ctx.enter_context(tc.tile_pool(name="ps_av", bufs=2, space="PSUM"))

            wqt_sb = singles.tile([128, 2, 512], f32r)
            nc.sync.dma_start(out=wqt_sb[:], in_=wqt_d.ap().rearrange("(c p) o -> p c o", p=128))
            wkt_sb = singles.tile([128, 2, 512], f32r)
            nc.sync.dma_start(out=wkt_sb[:], in_=wkt_d.ap().rearrange("(c p) o -> p c o", p=128))
            wvt_sb = singles.tile([128, 4, 256], f32r)
            nc.sync.dma_start(out=wvt_sb[:], in_=wvt_d.ap().rearrange("k p o -> p k o"))
            bq_sb = singles.tile([128, 4], f32)
            nc.sync.dma_start(out=bq_sb[:], in_=bq_d.ap().rearrange("(o p) -> p o", p=128))
            bk_sb = singles.tile([128, 4], f32)
            nc.sync.dma_start(out=bk_sb[:], in_=bk_d.ap().rearrange("(o p) -> p o", p=128))

            q_sb = bigs.tile([128, 4, S], bf16)
            k_sb = bigs.tile([128, 4, S], bf16)
            zh_sb = bigs.tile([128, S], bf16)   # one head-pair (128 channels) at a time

            def attn_strip(t, e_lhs, e_rhs, vt_strip, z_out_ap):
                """One head t, NJ j's. e_lhs/e_rhs: per-j [128,96] APs (K-slice, Q-slice).
                vt_strip: [96, NJ, 320] bf16 (cols 256:320 ones). z_out_ap: [64, NJ, 96] bf16."""
                e_ps = ps_e.tile([96, NJ, 96], mybir.dt.float32, tag="eps")
                for j in range(NJ):
                    nc.tensor.matmul(e_ps[:, j, :], e_lhs[j], e_rhs[j], start=True, stop=True)
                u_sb = strips.tile([96, NJ, 96], bf16, tag="u")
                nc.scalar.activation(out=u_sb[:], in_=e_ps[:],
                                     func=mybir.ActivationFunctionType.Exp, scale=SCALE)
                av_ps = ps_av.tile([128, NJ, 96], mybir.dt.float32, tag="avps")
                for j in range(NJ):
                    nc.tensor.matmul(av_ps[:, j, :], vt_strip[:, j, t, :], u_sb[:, j, :],
                                     start=True, stop=True)
                rec = strips.tile([64, NJ, 96], mybir.dt.float32, tag="rec")
                nc.vector.reciprocal(out=rec[:], in_=av_ps[64:128, :, :])
                nc.vector.tensor_tensor(out=z_out_ap, in0=av_ps[0:64, :, :], in1=rec[:],
                                        op=AluOpType.mult)

            def load_vt_strip(in_ap_fn):
                # [96, NJ, 4 heads, 128]: per head cols 0:64 = V^T, 64:128 = ones
                vt_strip = strips.tile([96, NJ, 4, 128], bf16, tag="vts")
                nc.sync.dma_start(
                    out=vt_strip[:].rearrange("p a b c -> p (a b) c")[:, :, 64:128],
                    in_=bass.AP(tensor=ones_d, offset=0, ap=[[0, 96], [0, NJ * 4], [1, 64]]))
                if in_ap_fn is not None:
                    for j in range(NJ):
                        nc.sync.dma_start(out=vt_strip[:, j, :, 0:64], in_=in_ap_fn(j))
                return vt_strip

            for b in range(BLOC):
                xrow = b * CIN
                # ============ PHASE A: convs + vertical ("out_w") attention ============
                for st in range(NST):
                    c0 = st * ST
                    x_t = []
                    for cc in range(4):
                        xt = xs.tile([128, ST], f32r, tag=f"x{cc}")
                        nc.sync.dma_start(out=xt[:],
                                          in_=x_d.ap()[xrow + cc * 128: xrow + (cc + 1) * 128, c0:c0 + ST])
                        x_t.append(xt)
                    for (w_sb, b_sb, dst, isq) in ((wqt_sb, bq_sb, q_sb, True), (wkt_sb, bk_sb, k_sb, False)):
                        for oct in range(4):
                            g = oct // 2
                            cps = ps_conv.tile([128, ST], mybir.dt.float32, tag="conv")
                            for cc in range(2):
                                nc.tensor.matmul(
                                    cps[:], w_sb[:, cc, oct * 128:(oct + 1) * 128], x_t[g * 2 + cc][:],
                                    start=(cc == 0), stop=(cc == 1))
                            if isq:
                                nc.scalar.activation(out=dst[:, oct, c0:c0 + ST], in_=cps[:],
                                                     func=mybir.ActivationFunctionType.Identity,
                                                     bias=b_sb[:, oct:oct + 1], scale=1.0)
                            else:
                                nc.vector.tensor_scalar(out=dst[:, oct, c0:c0 + ST], in0=cps[:],
                                                        scalar1=b_sb[:, oct:oct + 1], scalar2=None,
                                                        op0=AluOpType.add)
                    vt_strip = load_vt_strip(None)
                    for j in range(NJ):
                        vps = ps_vt.tile([96, 256], mybir.dt.float32, tag="vt")
                        for cc in range(4):
                            nc.tensor.matmul(vps[:], x_t[cc][:, j * 96:(j + 1) * 96], wvt_sb[:, cc, :],
                                             start=(cc == 0), stop=(cc == 3))
                        nc.scalar.copy(out=vt_strip[:, j, :, 0:64],
                                       in_=vps[:].rearrange("p (t c) -> p t c", c=64))
                    for j in range(NJ):
                        nc.sync.dma_start(
                            out=bass.AP(tensor=vt_scr, offset=(c0 + j * 96) * 256,
                                        ap=[[256, 96], [64, 4], [1, 64]]),
                            in_=vt_strip[:, j, :, 0:64])
                    z_strip = strips.tile([128, 2, NJ, 96], bf16, tag="zs")
                    for t in range(4):
                        lhs = [k_sb[:, t, c0 + j * 96: c0 + (j + 1) * 96] for j in range(NJ)]
                        rhs = [q_sb[:, t, c0 + j * 96: c0 + (j + 1) * 96] for j in range(NJ)]
                        attn_strip(t, lhs, rhs, vt_strip,
                                   z_strip[(t % 2) * 64:(t % 2) * 64 + 64, t // 2, :, :])
                    for zt in range(2):
                        o_t = outs.tile([128, ST], bf16, tag="ot")
                        nc.vector.scalar_tensor_tensor(
                            out=o_t[:], in0=z_strip[:, zt, :, :].rearrange("p a b -> p (a b)"),
                            scalar=0.0, in1=x_t[2 + zt][:].bitcast(f32),
                            op0=AluOpType.max, op1=AluOpType.add)
                        nc.sync.dma_start(
                            out=out_d.ap()[xrow + 256 + zt * 128: xrow + 256 + (zt + 1) * 128, c0:c0 + ST],
                            in_=o_t[:])
                    nc.sync.dma_start(out=zr_d.ap()[b * 256 + 128: b * 256 + 256, c0:c0 + ST],
                                      in_=z_strip[:, 0, :, :].rearrange("p a b -> p (a b)"))

                # ============ PHASE B: horizontal ("out_h") attention ============
                qr = q_sb[:].rearrange("p o (h w) -> p o w h", w=96)
                kr = k_sb[:].rearrange("p o (h w) -> p o w h", w=96)
                zhr = zh_sb[:].rearrange("p (h w) -> p w h", w=96)
                for zt in range(2):
                    for wst in range(NST):
                        w0 = wst * NJ
                        vt_strip = load_vt_strip(
                            lambda j, w0=w0: bass.AP(tensor=vt_scr, offset=(w0 + j) * 256,
                                                     ap=[[96 * 256, 96], [64, 4], [1, 64]]))
                        for t in (2 * zt, 2 * zt + 1):
                            lhs = [kr[:, t, w0 + j, :] for j in range(NJ)]
                            rhs = [qr[:, t, w0 + j, :] for j in range(NJ)]
                            attn_strip(t, lhs, rhs, vt_strip,
                                       zhr[(t % 2) * 64:(t % 2) * 64 + 64, w0:w0 + NJ, :])
                    for ch in range(S // NEC):
                        e0 = ch * NEC
                        xe = outs.tile([128, NEC], f32r, tag="xe")
                        nc.sync.dma_start(out=xe[:],
                                          in_=x_d.ap()[xrow + zt * 128: xrow + (zt + 1) * 128, e0:e0 + NEC])
                        o_t = outs.tile([128, NEC], bf16, tag="oe")
                        nc.vector.scalar_tensor_tensor(
                            out=o_t[:], in0=zh_sb[:, e0:e0 + NEC], scalar=0.0,
                            in1=xe[:].bitcast(f32), op0=AluOpType.max, op1=AluOpType.add)
                        nc.sync.dma_start(out=out_d.ap()[xrow + zt * 128: xrow + (zt + 1) * 128, e0:e0 + NEC],
                                          in_=o_t[:])
                    if zt == 0:
                        nc.sync.dma_start(out=zr_d.ap()[b * 256: b * 256 + 128, :], in_=zh_sb[:])

    nc.finalize()
    return nc


def kernel(x, wq, bq, wk, bk, wv, bv, bnh_w, bnh_b, bnw_w, bnw_b, gamma, bn_rm, bn_rv):
    global _COMPILED
    from concourse.bass_utils import run_bass_kernel_spmd

    x = np.asarray(x, np.float32)
    wq = np.asarray(wq, np.float32); wk = np.asarray(wk, np.float32); wv = np.asarray(wv, np.float32)
    bq = np.asarray(bq, np.float32); bk = np.asarray(bk, np.float32); bv = np.asarray(bv, np.float32)
    g = float(np.asarray(gamma).reshape(-1)[0])
    assert g > 0, "kernel assumes gamma > 0"
    assert np.all(bv == 0), "kernel assumes zero v-bias"

    if _COMPILED is None:
        _COMPILED = _build()
    nc = _COMPILED

    wvt_pad = np.zeros((4, 128, 256), np.float32)
    wv_g = g * wv
    for k in range(4):
        grp, cc = k // 2, k % 2
        blk = wv_g[grp * 128:(grp + 1) * 128, cc * 128:(cc + 1) * 128]
        wvt_pad[k, :, grp * 128:(grp + 1) * 128] = blk.T

    base = {
        "wqt": np.ascontiguousarray(wq.T),
        "wkt": np.ascontiguousarray(wk.T),
        "wvt": wvt_pad,
        "bq": bq, "bk": bk,
        "ones64": np.ones(64, BF16),
    }
    in_maps = []
    for i in range(NCORES):
        m = dict(base)
        m["x"] = np.ascontiguousarray(x[i * BLOC:(i + 1) * BLOC].reshape(BLOC * CIN, S))
        in_maps.append(m)

    res = run_bass_kernel_spmd(nc, in_maps, core_ids=list(range(NCORES)))

    out = np.empty((B, CIN, H, W), np.float32)
    p_h = np.empty((B, 128, H, W), np.float32)
    p_w = np.empty((B, 128, H, W), np.float32)
    bn_rv = np.asarray(bn_rv, np.float32); bn_rm = np.asarray(bn_rm, np.float32)
    inv_h = np.asarray(bnh_w, np.float32) / np.sqrt(bn_rv + EPS)
    bias_h = np.asarray(bnh_b, np.float32) - bn_rm * inv_h
    inv_w = np.asarray(bnw_w, np.float32) / np.sqrt(bn_rv + EPS)
    bias_w = np.asarray(bnw_b, np.float32) - bn_rm * inv_w
    sh = (inv_h[:128] / g)[:, None, None]
    sw = (inv_w[:128] / g)[:, None, None]
    for i in range(NCORES):
        r = res.results[i]
        out[i * BLOC:(i + 1) * BLOC] = np.asarray(r["out"]).astype(np.float32).reshape(BLOC, CIN, H, W)
        zr = np.asarray(r["zr"]).astype(np.float32).reshape(BLOC, 2, 128, H, W)
        for b in range(BLOC):
            p_h[i * BLOC + b] = zr[b, 0] * sh + bias_h[:128, None, None]
            p_w[i * BLOC + b] = zr[b, 1] * sw + bias_w[:128, None, None]
    return out, p_h, p_w


# revision 5
# speedup vs baseline: 210.8479x; 210.8479x over previous
import sys
import numpy as np

sys.path.insert(0, '/opt/trn_rl_repo')
import ml_dtypes  # noqa: E402

BF16 = ml_dtypes.bfloat16

B, CIN, H, W = 16, 512, 96, 96
HEAD = 4
QK = 512
CV = 256
S = H * W            # 9216
SCALE = (QK // HEAD) ** -0.5
EPS = 1e-5
NCORES = 8
BLOC = B // NCORES   # 2 images per core
ST = 288             # spatial tile = 3 rows of 96
NJ = 3               # attention j's per strip
NST = S // ST        # 32
NEC = 576            # epilogue chunk (phase B)

_COMPILED = None
_last_in_maps = None


def _build():
    import concourse.bass as bass
    import concourse.mybir as mybir
    import concourse.tile as tile
    import concourse.tile_utils as tile_utils
    from concourse import bacc
    from concourse.alu_op_type import AluOpType

    tile_utils.max_sbuf_usage = 206 * 1024

    f32 = mybir.dt.float32
    f32r = mybir.dt.float32r
    bf16 = mybir.dt.bfloat16

    nc = bacc.Bacc("TRN2", target_bir_lowering=False, debug=False, enable_asserts=False)

    x_d = nc.dram_tensor("x", [BLOC * CIN, S], f32r, kind="ExternalInput")
    wqt_d = nc.dram_tensor("wqt", [256, 512], f32r, kind="ExternalInput")
    wkt_d = nc.dram_tensor("wkt", [256, 512], f32r, kind="ExternalInput")
    wvt_d = nc.dram_tensor("wvt", [4, 128, 256], f32r, kind="ExternalInput")
    bq_d = nc.dram_tensor("bq", [512], f32, kind="ExternalInput")
    bk_d = nc.dram_tensor("bk", [512], f32, kind="ExternalInput")
    ones_d = nc.dram_tensor("ones64", [64], bf16, kind="ExternalInput")
    out_d = nc.dram_tensor("out", [BLOC * CIN, S], bf16, kind="ExternalOutput")
    zr_d = nc.dram_tensor("zr", [BLOC * 256, S], bf16, kind="ExternalOutput")
    vt_scr = nc.dram_tensor("vt_scr", [S, 256], bf16)

    with tile.TileContext(nc) as tc:
        import contextlib
        with contextlib.ExitStack() as ctx:
            singles = ctx.enter_context(tc.tile_pool(name="singles", bufs=1))
            bigs = ctx.enter_context(tc.tile_pool(name="bigs", bufs=1))
            xs = ctx.enter_context(tc.tile_pool(name="xs", bufs=2))
            strips = ctx.enter_context(tc.tile_pool(name="strips", bufs=2))
            outs = ctx.enter_context(tc.tile_pool(name="outs", bufs=2))
            ps_conv = ctx.enter_context(tc.tile_pool(name="ps_conv", bufs=2, space="PSUM"))
            ps_vt = ctx.enter_context(tc.tile_pool(name="ps_vt", bufs=2, space="PSUM"))
            ps_e = ctx.enter_context(tc.tile_pool(name="ps_e", bufs=2, space="PSUM"))
            ps_av = ctx.enter_context(tc.tile_pool(name="ps_av", bufs=2, space="PSUM"))

            wqt_sb = singles.tile([128, 2, 512], f32r)
            nc.sync.dma_start(out=wqt_sb[:], in_=wqt_d.ap().rearrange("(c p) o -> p c o", p=128))
            wkt_sb = singles.tile([128, 2, 512], f32r)
            nc.sync.dma_start(out=wkt_sb[:], in_=wkt_d.ap().rearrange("(c p) o -> p c o", p=128))
            wvt_sb = singles.tile([128, 4, 256], f32r)
            nc.sync.dma_start(out=wvt_sb[:], in_=wvt_d.ap().rearrange("k p o -> p k o"))
            bq_sb = singles.tile([128, 4], f32)
            nc.sync.dma_start(out=bq_sb[:], in_=bq_d.ap().rearrange("(o p) -> p o", p=128))
            bk_sb = singles.tile([128, 4], f32)
            nc.sync.dma_start(out=bk_sb[:], in_=bk_d.ap().rearrange("(o p) -> p o", p=128))

            q_sb = bigs.tile([128, 4, S], bf16)
            k_sb = bigs.tile([128, 4, S], bf16)
            zh_sb = bigs.tile([128, S], bf16)   # one head-pair (128 channels) at a time

            def attn_strip(t, e_lhs, e_rhs, vt_strip, z_out_ap):
                """One head t, NJ j's. e_lhs/e_rhs: per-j [128,96] APs (K-slice, Q-slice).
                vt_strip: [96, NJ, 320] bf16 (cols 256:320 ones). z_out_ap: [64, NJ, 96] bf16."""
                e_ps = ps_e.tile([96, NJ, 96], mybir.dt.float32, tag="eps")
                for j in range(NJ):
                    nc.tensor.matmul(e_ps[:, j, :], e_lhs[j], e_rhs[j], start=True, stop=True)
                u_sb = strips.tile([96, NJ, 96], bf16, tag="u")
                nc.scalar.activation(out=u_sb[:], in_=e_ps[:],
                                     func=mybir.ActivationFunctionType.Exp, scale=SCALE)
                av_ps = ps_av.tile([128, NJ, 96], mybir.dt.float32, tag="avps")
                for j in range(NJ):
                    nc.tensor.matmul(av_ps[:, j, :], vt_strip[:, j, t, :], u_sb[:, j, :],
                                     start=True, stop=True)
                rec = strips.tile([64, NJ, 96], mybir.dt.float32, tag="rec")
                nc.vector.reciprocal(out=rec[:], in_=av_ps[64:128, :, :])
                nc.vector.tensor_tensor(out=z_out_ap, in0=av_ps[0:64, :, :], in1=rec[:],
                                        op=AluOpType.mult)

            def load_vt_strip(in_ap_fn):
                # [96, NJ, 4 heads, 128]: per head cols 0:64 = V^T, 64:128 = ones
                vt_strip = strips.tile([96, NJ, 4, 128], bf16, tag="vts")
                nc.sync.dma_start(
                    out=vt_strip[:].rearrange("p a b c -> p (a b) c")[:, :, 64:128],
                    in_=bass.AP(tensor=ones_d, offset=0, ap=[[0, 96], [0, NJ * 4], [1, 64]]))
                if in_ap_fn is not None:
                    for j in range(NJ):
                        nc.sync.dma_start(out=vt_strip[:, j, :, 0:64], in_=in_ap_fn(j))
                return vt_strip

            for b in range(BLOC):
                xrow = b * CIN
                # ============ PHASE A: convs + vertical ("out_w") attention ============
                for st in range(NST):
                    c0 = st * ST
                    x_t = []
                    for cc in range(4):
                        xt = xs.tile([128, ST], f32r, tag=f"x{cc}")
                        nc.sync.dma_start(out=xt[:],
                                          in_=x_d.ap()[xrow + cc * 128: xrow + (cc + 1) * 128, c0:c0 + ST])
                        x_t.append(xt)
                    for (w_sb, b_sb, dst, isq) in ((wqt_sb, bq_sb, q_sb, True), (wkt_sb, bk_sb, k_sb, False)):
                        for oct in range(4):
                            g = oct // 2
                            cps = ps_conv.tile([128, ST], mybir.dt.float32, tag="conv")
                            for cc in range(2):
                                nc.tensor.matmul(
                                    cps[:], w_sb[:, cc, oct * 128:(oct + 1) * 128], x_t[g * 2 + cc][:],
                                    start=(cc == 0), stop=(cc == 1))
                            if isq:
                                nc.scalar.activation(out=dst[:, oct, c0:c0 + ST], in_=cps[:],
                                                     func=mybir.ActivationFunctionType.Identity,
                                                     bias=b_sb[:, oct:oct + 1], scale=1.0)
                            else:
                                nc.vector.tensor_scalar(out=dst[:, oct, c0:c0 + ST], in0=cps[:],
                                                        scalar1=b_sb[:, oct:oct + 1], scalar2=None,
                                                        op0=AluOpType.add)
                    vt_strip = load_vt_strip(None)
                    for j in range(NJ):
                        vps = ps_vt.tile([96, 256], mybir.dt.float32, tag="vt")
                        for cc in range(4):
                            nc.tensor.matmul(vps[:], x_t[cc][:, j * 96:(j + 1) * 96], wvt_sb[:, cc, :],
                                             start=(cc == 0), stop=(cc == 3))
                        nc.scalar.copy(out=vt_strip[:, j, :, 0:64],
                                       in_=vps[:].rearrange("p (t c) -> p t c", c=64))
                    for j in range(NJ):
                        nc.sync.dma_start(
                            out=bass.AP(tensor=vt_scr, offset=(c0 + j * 96) * 256,
                                        ap=[[256, 96], [64, 4], [1, 64]]),
                            in_=vt_strip[:, j, :, 0:64])
                    z_strip = strips.tile([128, 2, NJ, 96], bf16, tag="zs")
                    for t in range(4):
                        lhs = [k_sb[:, t, c0 + j * 96: c0 + (j + 1) * 96] for j in range(NJ)]
                        rhs = [q_sb[:, t, c0 + j * 96: c0 + (j + 1) * 96] for j in range(NJ)]
                        attn_strip(t, lhs, rhs, vt_strip,
                                   z_strip[(t % 2) * 64:(t % 2) * 64 + 64, t // 2, :, :])
                    for zt in range(2):
                        o_t = outs.tile([128, ST], bf16, tag="ot")
                        nc.vector.scalar_tensor_tensor(
                            out=o_t[:], in0=z_strip[:, zt, :, :].rearrange("p a b -> p (a b)"),
                            scalar=0.0, in1=x_t[2 + zt][:].bitcast(f32),
                            op0=AluOpType.max, op1=AluOpType.add)
                        nc.sync.dma_start(
                            out=out_d.ap()[xrow + 256 + zt * 128: xrow + 256 + (zt + 1) * 128, c0:c0 + ST],
                            in_=o_t[:])
                    nc.sync.dma_start(out=zr_d.ap()[b * 256 + 128: b * 256 + 256, c0:c0 + ST],
                                      in_=z_strip[:, 0, :, :].rearrange("p a b -> p (a b)"))

                # ============ PHASE B: horizontal ("out_h") attention ============
                qr = q_sb[:].rearrange("p o (h w) -> p o w h", w=96)
                kr = k_sb[:].rearrange("p o (h w) -> p o w h", w=96)
                zhr = zh_sb[:].rearrange("p (h w) -> p w h", w=96)
                for zt in range(2):
                    for wst in range(NST):
                        w0 = wst * NJ
                        vt_strip = load_vt_strip(
                            lambda j, w0=w0: bass.AP(tensor=vt_scr, offset=(w0 + j) * 256,
                                                     ap=[[96 * 256, 96], [64, 4], [1, 64]]))
                        for t in (2 * zt, 2 * zt + 1):
                            lhs = [kr[:, t, w0 + j, :] for j in range(NJ)]
                            rhs = [qr[:, t, w0 + j, :] for j in range(NJ)]
                            attn_strip(t, lhs, rhs, vt_strip,
                                       zhr[(t % 2) * 64:(t % 2) * 64 + 64, w0:w0 + NJ, :])
                    for ch in range(S // NEC):
                        e0 = ch * NEC
                        xe = outs.tile([128, NEC], f32r, tag="xe")
                        nc.sync.dma_start(out=xe[:],
                                          in_=x_d.ap()[xrow + zt * 128: xrow + (zt + 1) * 128, e0:e0 + NEC])
                        o_t = outs.tile([128, NEC], bf16, tag="oe")
                        nc.vector.scalar_tensor_tensor(
                            out=o_t[:], in0=zh_sb[:, e0:e0 + NEC], scalar=0.0,
                            in1=xe[:].bitcast(f32), op0=AluOpType.max, op1=AluOpType.add)
                        nc.sync.dma_start(out=out_d.ap()[xrow + zt * 128: xrow + (zt + 1) * 128, e0:e0 + NEC],
                                          in_=o_t[:])
                    if zt == 0:
                        nc.sync.dma_start(out=zr_d.ap()[b * 256: b * 256 + 128, :], in_=zh_sb[:])

    nc.finalize()
    return nc


def kernel(x, wq, bq, wk, bk, wv, bv, bnh_w, bnh_b, bnw_w, bnw_b, gamma, bn_rm, bn_rv):
    global _COMPILED
    from concourse.bass_utils import run_bass_kernel_spmd

    x = np.asarray(x, np.float32)
    wq = np.asarray(wq, np.float32); wk = np.asarray(wk, np.float32); wv = np.asarray(wv, np.float32)
    bq = np.asarray(bq, np.float32); bk = np.asarray(bk, np.float32); bv = np.asarray(bv, np.float32)
    g = float(np.asarray(gamma).reshape(-1)[0])
    assert g > 0, "kernel assumes gamma > 0"
    assert np.all(bv == 0), "kernel assumes zero v-bias"

    if _COMPILED is None:
        _COMPILED = _build()
    nc = _COMPILED

    wvt_pad = np.zeros((4, 128, 256), np.float32)
    wv_g = g * wv
    for k in range(4):
        grp, cc = k // 2, k % 2
        blk = wv_g[grp * 128:(grp + 1) * 128, cc * 128:(cc + 1) * 128]
        wvt_pad[k, :, grp * 128:(grp + 1) * 128] = blk.T

    base = {
        "wqt": np.ascontiguousarray(wq.T),
        "wkt": np.ascontiguousarray(wk.T),
        "wvt": wvt_pad,
        "bq": bq, "bk": bk,
        "ones64": np.ones(64, BF16),
    }
    in_maps = []
    for i in range(NCORES):
        m = dict(base)
        m["x"] = np.ascontiguousarray(x[i * BLOC:(i + 1) * BLOC].reshape(BLOC * CIN, S))
        in_maps.append(m)

    global _last_in_maps
    _last_in_maps = in_maps
    res = run_bass_kernel_spmd(nc, in_maps, core_ids=list(range(NCORES)))

    out = np.empty((B, CIN, H, W), np.float32)
    p_h = np.empty((B, 128, H, W), np.float32)
    p_w = np.empty((B, 128, H, W), np.float32)
    bn_rv = np.asarray(bn_rv, np.float32); bn_rm = np.asarray(bn_rm, np.float32)
    inv_h = np.asarray(bnh_w, np.float32) / np.sqrt(bn_rv + EPS)
    bias_h = np.asarray(bnh_b, np.float32) - bn_rm * inv_h
    inv_w = np.asarray(bnw_w, np.float32) / np.sqrt(bn_rv + EPS)
    bias_w = np.asarray(bnw_b, np.float32) - bn_rm * inv_w
    sh = (inv_h[:128] / g)[:, None, None]
    sw = (inv_w[:128] / g)[:, None, None]
    for i in range(NCORES):
        r = res.results[i]
        out[i * BLOC:(i + 1) * BLOC] = np.asarray(r["out"]).astype(np.float32).reshape(BLOC, CIN, H, W)
        zr = np.asarray(r["zr"]).astype(np.float32).reshape(BLOC, 2, 128, H, W)
        for b in range(BLOC):
            p_h[i * BLOC + b] = zr[b, 0] * sh + bias_h[:128, None, None]
            p_w[i * BLOC + b] = zr[b, 1] * sw + bias_w[:128, None, None]
    return out, p_h, p_w
